# revision 26
# baseline (speedup 1.0000x reference)
"""Bass/Trainium2 kernel for nn_EnergyModel (3-layer GAT + MLP head).

Sharding: data-parallel over batch B=32 across 8 NeuronCores (4 graphs/core),
GAT/MLP params replicated.

Per-graph algorithm (3 GAT layers); graphs are processed in interleaved PAIRS
(g0.L0, g1.L0, g0.L1, g1.L1, ...) so each layer's serial dependency chain
(sd -> gather -> S -> exp -> agg -> normalize) overlaps the sibling graph's
matmul work and the PE stays busy/warm:
  - atomsT [c_in, 256] kept transposed (c on partitions).
  - h   = atoms @ W         -> PE, out [i, (r,c)] psum -> sbuf
  - hT  blocks [c, i] per r -> PE (lhsT = W r-slice)
  - src/dst = a . h         -> PE (lhsT = Asel block-diag [c, 2R])
  - S[(r,jh)-block j', i] = dst_j + src_i + BIG*(bond-1), all accumulated in
    PSUM: rank-2 matmul with augmented [dst|ones] x [ones|src] tiles, plus
    per-i-chunk mask matmuls  Am[i,(r,jh,j')]^T @ (BIG*I)  -- the matmul with
    identity rhs transposes the bond mask on the fly, so the mask never needs
    a DMA/PE transpose pass nor a separate add pass.
  - L = leaky(S) (ACT Prelu, PSUM->SBUF), Et = Exp(L) (ACT).
  - out^T[c, i] = sum_b h_b^T Et_b (PE, k=1280 accumulation)
  - Z[1, i] = ones^T Et (PE), rz = 1/Z (fast approx recip), broadcast by
    rank-1 matmul, atomsT_next = leaky(out^T) * rz.
  - layer 3: y_feats from mean/max over i; MLP head batched over 4 graphs.
"""

import sys
from contextlib import ExitStack

if "/opt/trn_rl_repo" not in sys.path:
    sys.path.insert(0, "/opt/trn_rl_repo")

import numpy as np

B, N, CIN, C, R, XD = 32, 256, 64, 128, 5, 1024
NCORE = 8
NG = B // NCORE  # graphs per core
NRC = R * C      # 640
H1 = 256         # MLP hidden 1
H2 = 32          # MLP hidden 2
ZDIM = 2 * C + XD  # 1280
BIG = 3.0e38

_BUILD_CACHE = {}


def build(n_graphs=NG, with_bias=True, repeat=1):
    key = (n_graphs, with_bias, repeat)
    if key in _BUILD_CACHE:
        return _BUILD_CACHE[key]

    import concourse.bass as bass
    from concourse import bacc
    import concourse.tile as tile
    import concourse.mybir as mybir
    from concourse.masks import make_identity

    f32 = mybir.dt.float32
    f32r = mybir.dt.float32r
    bf16 = mybir.dt.bfloat16
    i32 = mybir.dt.int32
    AF = mybir.ActivationFunctionType
    OP = mybir.AluOpType

    def mm(out, lhsT, rhs, **kw):
        nc.tensor.matmul(out, lhsT, rhs, **kw)

    nc = bacc.Bacc("TRN2", target_bir_lowering=False)
    ng = n_graphs

    atoms_d = nc.dram_tensor("y_atoms", [ng, N, CIN], f32, kind="ExternalInput")
    bonds_d = nc.dram_tensor("y_bonds", [ng, N, N, R], i32, kind="ExternalInput")
    x_d = nc.dram_tensor("x", [ng, XD], f32, kind="ExternalInput")
    W_d = [
        nc.dram_tensor("W1", [CIN, NRC], f32, kind="ExternalInput"),
        nc.dram_tensor("W2", [C, NRC], f32, kind="ExternalInput"),
        nc.dram_tensor("W3", [C, NRC], f32, kind="ExternalInput"),
    ]
    a_d = [
        nc.dram_tensor(f"a{i}", [R, 2 * C], f32, kind="ExternalInput")
        for i in (1, 2, 3)
    ]
    We1_d = nc.dram_tensor("We1", [ZDIM, H1], f32, kind="ExternalInput")
    We2_d = nc.dram_tensor("We2", [H1, H2], f32, kind="ExternalInput")
    We3_d = nc.dram_tensor("We3", [H2, 1], f32, kind="ExternalInput")
    if with_bias:
        b_d = [
            nc.dram_tensor(f"b{i}", [1, NRC], f32, kind="ExternalInput")
            for i in (1, 2, 3)
        ]
        be1_d = nc.dram_tensor("be1", [1, H1], f32, kind="ExternalInput")
        be2_d = nc.dram_tensor("be2", [1, H2], f32, kind="ExternalInput")
        be3_d = nc.dram_tensor("be3", [1, 1], f32, kind="ExternalInput")
    out_d = nc.dram_tensor("out", [ng, 1], f32, kind="ExternalOutput")

    with tile.TileContext(nc) as tc, ExitStack() as ctx:
        const = ctx.enter_context(tc.tile_pool(name="const", bufs=1))
        gpool = ctx.enter_context(tc.tile_pool(name="gpool", bufs=2))
        gpool3 = ctx.enter_context(tc.tile_pool(name="gpool3", bufs=1))
        spool = ctx.enter_context(tc.tile_pool(name="spool", bufs=2))
        ps_s = ctx.enter_context(tc.tile_pool(name="ps_s", bufs=2, space="PSUM"))
        ps_sm = ctx.enter_context(tc.tile_pool(name="ps_sm", bufs=4, space="PSUM"))

        # ---------------- constants ----------------
        ident = const.tile([128, 128], f32)
        make_identity(nc, ident[:])
        # BIG * identity in bf16: rhs of the mask-transpose matmuls
        bigI = const.tile([128, 128], bf16)
        nc.vector.tensor_scalar(bigI[:], ident[:], BIG, None, op0=OP.mult)
        onesf = const.tile([128, 1], f32)
        nc.vector.memset(onesf[:], 1.0)
        ones_col = const.tile([128, 1], f32r)
        nc.vector.tensor_copy(ones_col[:], onesf[:])
        ones_col_bf = const.tile([128, 1], bf16)
        nc.vector.tensor_copy(ones_col_bf[:], onesf[:])
        onesrf = const.tile([1, 256], f32)
        nc.vector.memset(onesrf[:], 1.0)
        ones_row = const.tile([1, 256], f32r)
        nc.vector.tensor_copy(ones_row[:], onesrf[:])
        ones_row_bf = const.tile([1, 256], bf16)
        nc.vector.tensor_copy(ones_row_bf[:], onesrf[:])

        W_sb = []
        for li in range(3):
            cin = CIN if li == 0 else C
            w_raw = spool.tile([cin, NRC], f32, tag="w_raw")
            nc.sync.dma_start(w_raw[:], W_d[li][:])
            w = const.tile([cin, NRC], bf16, tag=f"W{li}")
            nc.vector.tensor_copy(w[:], w_raw[:])
            W_sb.append(w)

        # Asel[l]: [c, r, m] block-diagonal src/dst selector: column m=r of
        # k-chunk r holds the src half a[r, c]; column m=R+r the dst half
        # a[r, C+c]; other columns zero. One accumulated matmul over the 5
        # k-chunks then yields sd[m, i].
        # MLP lhsT staging: z^T chunks [128, kb, g]; kb 0..7 = x, 8 = mean, 9 = max
        zT = const.tile([128, 10, ng], f32r)

        # ---------------- per-graph pieces ----------------
        def prep_graph(g, lane):
            st = {}
            # atoms load -> bf16 -> PE transpose blocks -> atomsT [c, i]
            at_nat = spool.tile([128, 2, CIN], f32, tag=f"atnat_{lane}")
            for ib in range(2):
                nc.sync.dma_start(at_nat[:, ib, :],
                                  atoms_d[g, ib * 128:(ib + 1) * 128, :])
            atT_ps = ps_sm.tile([CIN, 2, 128], f32, tag="sm")
            for ib in range(2):
                nc.tensor.matmul(
                    atT_ps[:, ib, :], at_nat[:, ib, :], ident[:],
                    is_transpose=True, start=True, stop=True,
                )
            atoms_cur = gpool.tile([CIN, 256], bf16, tag=f"atoms0_{lane}")
            nc.vector.tensor_copy(
                atoms_cur[:], atT_ps.rearrange("c a b -> c (a b)")
            )
            st["atoms"] = atoms_cur

            # bonds -> Am[ib] = (bond - 1) in bf16, r-major free layout
            # Am[ib][i', r, jh, j']; lhsT slices Am[ib][:, r, jh, :] are the
            # mask-transpose matmul stationary operands.
            Am = []
            for ib in range(2):
                bonds_sb = gpool.tile([128, N * R], i32, tag=f"bonds_{lane}")
                nc.gpsimd.dma_start(
                    bonds_sb[:],
                    bonds_d[g, ib * 128:(ib + 1) * 128].rearrange("p j r -> p (j r)"),
                )
                am = gpool.tile([128, R, 2, 128], bf16, tag=f"am{ib}_{lane}")
                bview = bonds_sb.rearrange("p (jh j r) -> p r jh j",
                                           jh=2, j=128, r=R)
                # split so the first S chunk (r=0,1) unblocks early
                nc.vector.tensor_scalar(am[:, 0:2], bview[:, 0:2], 1, None,
                                        op0=OP.subtract)
                nc.vector.tensor_scalar(am[:, 2:R], bview[:, 2:R], 1, None,
                                        op0=OP.subtract)
                Am.append(am)
            st["Am"] = Am

            # x staging for MLP (stage f32, round-copy into f32r zT)
            x_stage = spool.tile([128, 8], f32, tag=f"xstage_{lane}")
            nc.sync.dma_start(x_stage[:], x_d[g].rearrange("(f p) -> p f", p=128))
            nc.vector.tensor_copy(zT[:, 0:8, g:g + 1].rearrange("p a b -> p (a b)"),
                                  x_stage[:])
            return st

        def layer_ph1(lane, li, st):
            W = W_sb[li]
            atoms_cur = st["atoms"]
            Am = st["Am"]
            # h = atoms @ W (+b): out [i, (r,c)] in two n-chunks per i-block
            h_sb = gpool3.tile([128, 2, NRC], bf16, tag=f"h_{lane}")
            for ib in range(2):
                hA = ps_sm.tile([128, 384], f32, tag="sm")
                hB = ps_sm.tile([128, 256], f32, tag="sm")
                lt = atoms_cur[:, ib * 128:(ib + 1) * 128]
                mm(hA[:], lt, W[:, 0:384], start=True, stop=not with_bias)
                mm(hB[:], lt, W[:, 384:NRC], start=True, stop=not with_bias)
                if with_bias:
                    mm(hA[:], ones_row[:, :128], b_row[li][:, 0:384],
                       start=False, stop=True)
                    mm(hB[:], ones_row[:, :128], b_row[li][:, 384:NRC],
                       start=False, stop=True)
                if ib == 0:
                    nc.scalar.activation(h_sb[:, ib, 0:384], hA[:], AF.Copy)
                    nc.vector.tensor_copy(h_sb[:, ib, 384:NRC], hB[:])
                else:
                    nc.vector.tensor_copy(h_sb[:, ib, 0:384], hA[:])
                    nc.scalar.activation(h_sb[:, ib, 384:NRC], hB[:], AF.Copy)
            st["h_sb"] = h_sb

            # hT blocks: [c, i] per r (lhsT = W r-slice); 2-bank psum tiles
            hT_sb = gpool3.tile([128, R, 256], bf16, tag=f"ht_{lane}")
            for rp in range(3):
                rr = (2, 2, 1)[rp]
                r0 = 2 * rp
                hT_ps = ps_sm.tile([128, 2, 256], f32, tag="sm")
                for dr in range(rr):
                    r = r0 + dr
                    mm(hT_ps[:, dr, :], W[:, r * 128:(r + 1) * 128],
                       atoms_cur[:], start=True, stop=True)
                if with_bias:
                    for dr in range(rr):
                        r = r0 + dr
                        nc.scalar.activation(
                            hT_sb[:, r, :], hT_ps[:, dr, :],
                            AF.Prelu, bias=bcol[li][:, r:r + 1], alpha=1.0,
                        )
                else:
                    nc.vector.tensor_copy(
                        hT_sb[:, r0:r0 + rr, :].rearrange("p a b -> p (a b)"),
                        hT_ps[:, 0:rr, :].rearrange("p a b -> p (a b)"),
                    )

            # src/dst: sd_ps[2r+s, i] via accumulated block-diag matmul
            sd_ps = ps_sm.tile([2 * R, 256], f32, tag="sm")
            for r in range(R):
                mm(sd_ps[:], Asel_sb[li][:, r, :], hT_sb[:, r, :],
                   start=(r == 0), stop=(r == R - 1))
            # evict sd rows as compensated bf16 hi/lo pairs (base 32:
            # src rows 32-36, dst rows 37-41)
            sdhi = spool.tile([42, 256], bf16, tag=f"sdhi_{lane}")
            nc.vector.tensor_copy(sdhi[32:42, :], sd_ps[:])
            sdlo = spool.tile([42, 256], bf16, tag=f"sdlo_{lane}")
            nc.vector.tensor_tensor(sdlo[32:42, :], sd_ps[:], sdhi[32:42, :],
                                    op=OP.subtract)
            # gather into the aug tiles (ones rows pre-set once globally)
            dstP, srcP = augs[(lane, li % 2)]
            nc.sync.dma_start(dstP[0:1], sdhi[32 + R:32 + 2 * R, :])
            nc.sync.dma_start(dstP[1:2], sdlo[32 + R:32 + 2 * R, :])
            nc.gpsimd.dma_start(srcP[2:3], sdhi[32:32 + R, :])
            nc.gpsimd.dma_start(srcP[3:4], sdlo[32:32 + R, :])

        def layer_ph2(lane, li, st):
            W = W_sb[li]
            Am = st["Am"]
            h_sb = st["h_sb"]
            dstP, srcP = augs[(lane, li % 2)]
            # S blocks: mask matmuls (transpose-on-the-fly) + rank-2 aug
            # matmul, accumulated in PSUM; then leaky (ACT) + exp (ACT);
            # aggregation matmuls chase each exp chunk.
            Et = gpool3.tile([128, 10, 256], bf16, tag=f"et_{lane}")
            # o/z accumulate across chunks with start=False everywhere:
            # another tile's start=True clears has_written for its whole
            # 2KB PSUM bank, so an in-flight start-based accumulation can be
            # silently converted to overwrite.  Explicit zeroing + pure
            # accumulation is immune (hw=0 -> overwrite zeros, hw=1 -> +=0).
            o_ps = ps_sm.tile([128, 256], f32, tag="sm")
            nc.scalar.mul(o_ps[:], o_ps[:], 0.0)
            z_ps = ps_sm.tile([1, 256], f32, tag="sm")
            nc.vector.memset(z_ps[:], 0.0)
            b0 = 0
            for nb in (4, 4, 2):
                L_sb = gpool3.tile([128, 4, 256], f32, tag=f"lsb_{lane}")
                S_ps = ps_s.tile([128, 4, 256], f32, tag="sps")
                # PSUM has_written semantics (HW-verified): start=True
                # clears the whole 2KB bank, so exactly ONE start per bank
                # (k=0 and k=2; each k-slice is half a bank).  All other MMs
                # use start=False: first touch of a region overwrites (bits
                # cleared), later touches accumulate.  Mask matmuls go first
                # so the PE streams while the sd gather DMAs land.
                for k in range(nb):
                    b = b0 + k
                    r, jh = b // 2, b % 2
                    mm(S_ps[:, k, 0:128], Am[0][:, r, jh, :], bigI[:],
                       start=(k % 2 == 0), stop=False)
                    mm(S_ps[:, k, 128:256], Am[1][:, r, jh, :], bigI[:],
                       start=False, stop=False)
                for k in range(nb):
                    b = b0 + k
                    r, jh = b // 2, b % 2
                    mm(S_ps[:, k, :],
                       dstP[:, r, jh * 128:(jh + 1) * 128],
                       srcP[:, r, :],
                       start=False, stop=True)
                # L = leaky(S) on ACT (Prelu alpha=0.2), PSUM -> SBUF
                nc.scalar.activation(
                    L_sb[:, 0:nb].rearrange("p a b -> p (a b)"),
                    S_ps[:, 0:nb].rearrange("p a b -> p (a b)"),
                    AF.Prelu, alpha=0.2,
                )
                # Et = exp(L)
                nc.scalar.activation(
                    Et[:, b0:b0 + nb].rearrange("p a b -> p (a b)"),
                    L_sb[:, 0:nb].rearrange("p a b -> p (a b)"),
                    AF.Exp,
                )
                # aggregation for this chunk's blocks (single start per
                # PSUM bank: only the very first matmul of each output)
                for k in range(nb):
                    b = b0 + k
                    r, jh = b // 2, b % 2
                    mm(o_ps[:], h_sb[:, jh, r * 128:(r + 1) * 128],
                       Et[:, b, :], start=False, stop=(b == 9))
                for k in range(nb):
                    b = b0 + k
                    mm(z_ps[:], ones_col_bf[:],
                       Et[:, b, :], start=False, stop=(b == 9))
                b0 += nb

            # normalize (+ inter-layer leaky); fast approx reciprocal
            rz_sb = spool.tile([1, 256], f32, tag=f"rz_{lane}")
            nc.vector.reciprocal_approx_fast(rz_sb[:], z_ps[:])
            rz_bf = spool.tile([1, 256], bf16, tag=f"rzb_{lane}")
            nc.vector.tensor_copy(rz_bf[:], rz_sb[:])
            rzb_ps = ps_sm.tile([128, 256], f32, tag="sm")
            mm(rzb_ps[:], ones_row_bf[:, :128], rz_bf[:],
               start=True, stop=True)
            O_sb = spool.tile([128, 256], f32, tag=f"osb_{lane}")
            if li < 2:
                nc.scalar.activation(O_sb[:], o_ps[:], AF.Prelu, alpha=0.2)
            else:
                nc.scalar.activation(O_sb[:], o_ps[:], AF.Copy)
            nxt = gpool.tile([C, 256], bf16, tag=f"atoms{li + 1}_{lane}")
            nc.vector.tensor_tensor(nxt[:], O_sb[:], rzb_ps[:], op=OP.mult)
            st["atoms"] = nxt

        def feats(g, st):
            # y_feats: mean/max over atoms (free dim of h3T [c, i])
            h3T = st["atoms"]
            mean_raw = spool.tile([128, 1], f32, tag="mean")
            nc.vector.tensor_reduce(mean_raw[:], h3T[:], axis=mybir.AxisListType.X,
                                    op=OP.add)
            nc.vector.tensor_scalar(zT[:, 8, g:g + 1], mean_raw[:], 1.0 / N, None,
                                    op0=OP.mult)
            nc.vector.tensor_reduce(zT[:, 9, g:g + 1], h3T[:],
                                    axis=mybir.AxisListType.X, op=OP.max)

        # ---------------- pair-interleaved schedule ----------------
        # pair-0 prep is emitted FIRST so its atoms/bonds DMAs head the
        # queues; the remaining constants (Asel, aug tiles, MLP weights)
        # are emitted after -- none of them gate the first layer matmuls.
        pairs = [list(range(p, min(p + 2, ng))) for p in range(0, ng, 2)]
        sts0 = {}
        for g in pairs[0]:
            sts0[g] = prep_graph(g, g % 2)

        Asel_sb = []
        for li in range(3):
            aT = spool.tile([C, R, 2], f32, tag="a_t")
            nc.sync.dma_start(aT[:], a_d[li].rearrange("r (s c) -> c r s", s=2))
            Asel = const.tile([C, R, 2 * R], bf16, tag=f"asel{li}")
            nc.vector.memset(Asel[:], 0.0)
            for s in range(2):
                for r in range(R):
                    nc.scalar.activation(
                        Asel[:, r, s * R + r:s * R + r + 1], aT[:, r, s:s + 1],
                        AF.Copy,
                    )
            Asel_sb.append(Asel)

        if with_bias:
            b_row = []
            bcol = []
            for li in range(3):
                braw = spool.tile([1, NRC], f32, tag="braw")
                nc.sync.dma_start(braw[:], b_d[li][:])
                br = const.tile([1, NRC], f32r, tag=f"brow{li}")
                nc.vector.tensor_copy(br[:], braw[:])
                b_row.append(br)
                # bias as [c, r] column for per-partition ACT bias on hT
                bc = const.tile([C, R], f32, tag=f"bcol{li}")
                nc.sync.dma_start(bc[:], b_d[li].rearrange("1 (r c) -> c r", r=R, c=C))
                bcol.append(bc)
            beraw = spool.tile([1, H1], f32, tag="beraw")
            nc.sync.dma_start(beraw[:], be1_d[:])
            be1_row = const.tile([1, H1], f32r)
            nc.vector.tensor_copy(be1_row[:], beraw[:])
            be2_row = const.tile([1, H2], f32)
            nc.sync.dma_start(be2_row[:], be2_d[:])
            be3_row = const.tile([1, 1], f32)
            nc.sync.dma_start(be3_row[:], be3_d[:])

        # aug tiles (per lane, double-buffered over layers): allocated once,
        # ones rows memset once; gathers only overwrite the data rows.
        augs = {}
        for lane in range(2):
            for db in range(2):
                # rank-4 compensated bf16: S = dsthi + dstlo + srchi + srclo
                # dstP rows [dsthi; dstlo; 1; 1], srcP rows [1; 1; srchi; srclo]
                dP = const.tile([4, R, 256], bf16, tag=f"dstp{db}_{lane}")
                sP = const.tile([4, R, 256], bf16, tag=f"srcp{db}_{lane}")
                nc.vector.memset(dP[:], 1.0)
                nc.vector.memset(sP[:], 1.0)
                augs[(lane, db)] = (dP, sP)


        first = True
        We_loaded = False
        for _rep in range(repeat):
         for pg in pairs:
            if first:
                sts = sts0
                first = False
            else:
                sts = {}
                for g in pg:
                    sts[g] = prep_graph(g, g % 2)
            for li in range(3):
                for g in pg:
                    layer_ph1(g % 2, li, sts[g])
                for g in pg:
                    layer_ph2(g % 2, li, sts[g])
            for g in pg:
                feats(g, sts[g])

            if not We_loaded:
                # MLP weights staged here: the DMAs+casts overlap the later
                # pairs' GAT compute instead of the startup-critical loads
                We_loaded = True
                We1_raw = const.tile([128, 10, H1], f32)
                nc.scalar.dma_start(We1_raw[:],
                                    We1_d.rearrange("(kb p) n -> p kb n", p=128))
                We1_sb = const.tile([128, 10, H1], f32r)
                nc.vector.tensor_copy(We1_sb[:], We1_raw[:])
                We2_sb = const.tile([128, 2, H2], f32)
                nc.scalar.dma_start(We2_sb[:],
                                    We2_d.rearrange("(kb p) n -> p kb n", p=128))
                We3_sb = const.tile([H2, 1], f32)
                nc.scalar.dma_start(We3_sb[:], We3_d[:])

            # MLP head for this pair (overlaps the next pair's GAT work)
            g0, npg = pg[0], len(pg)
            zz_ps = ps_sm.tile([npg, H1], f32, tag="sm")
            for kb in range(10):
                mm(zz_ps[:], zT[:, kb, g0:g0 + npg], We1_sb[:, kb, :],
                   start=(kb == 0), stop=(kb == 9) and not with_bias)
            if with_bias:
                mm(zz_ps[:], ones_row[:, :npg], be1_row[:], start=False, stop=True)
            zzl = spool.tile([npg, H1], f32, tag="zzl")
            nc.scalar.activation(zzl[:], zz_ps[:], AF.Prelu, alpha=0.2)
            zzT_ps = ps_sm.tile([128, 2, npg], f32, tag="sm")
            for hh in range(2):
                nc.tensor.matmul(zzT_ps[:, hh, :], zzl[:, hh * 128:(hh + 1) * 128],
                                 ident[:npg, :npg], is_transpose=True,
                                 start=True, stop=True)
            zzT_sb = spool.tile([128, 2, npg], f32, tag="zzt")
            nc.vector.tensor_copy(zzT_sb[:], zzT_ps[:])

            z2_ps = ps_sm.tile([npg, H2], f32, tag="sm")
            for hh in range(2):
                nc.tensor.matmul(z2_ps[:], zzT_sb[:, hh, :], We2_sb[:, hh, :],
                                 start=(hh == 0), stop=(hh == 1) and not with_bias)
            if with_bias:
                nc.tensor.matmul(z2_ps[:], onesrf[:, :npg], be2_row[:],
                                 start=False, stop=True)
            z2l = spool.tile([npg, H2], f32, tag="z2l")
            nc.scalar.activation(z2l[:], z2_ps[:], AF.Prelu, alpha=0.2)
            z2T_ps = ps_sm.tile([H2, npg], f32, tag="sm")
            nc.tensor.matmul(z2T_ps[:], z2l[:], ident[:npg, :npg],
                             is_transpose=True, start=True, stop=True)
            z2T_sb = spool.tile([H2, npg], f32, tag="z2t")
            nc.vector.tensor_copy(z2T_sb[:], z2T_ps[:])

            y_ps = ps_sm.tile([npg, 1], f32, tag="sm")
            nc.tensor.matmul(y_ps[:], z2T_sb[:], We3_sb[:], start=True,
                             stop=not with_bias)
            if with_bias:
                nc.tensor.matmul(y_ps[:], onesrf[:, :npg], be3_row[:],
                                 start=False, stop=True)
            y_sb = spool.tile([npg, 1], f32, tag="y")
            nc.vector.tensor_copy(y_sb[:], y_ps[:])
            nc.sync.dma_start(out_d[g0:g0 + npg], y_sb[:])

    nc.compile()
    _BUILD_CACHE[key] = nc
    return nc


_PARAM_KEYS = ("W1", "W2", "W3", "a1", "a2", "a3", "We1", "We2", "We3")
_BIAS_KEYS = ("b1", "b2", "b3", "be1", "be2", "be3")


def _shard_inputs(inputs, with_bias, n_cores, ng):
    per_core = []
    for c in range(n_cores):
        s = slice(c * ng, (c + 1) * ng)
        m = {
            "y_atoms": np.ascontiguousarray(inputs["y_atoms"][s], np.float32),
            "y_bonds": np.ascontiguousarray(inputs["y_bonds"][s], np.int32),
            "x": np.ascontiguousarray(inputs["x"][s], np.float32),
        }
        for k in _PARAM_KEYS:
            m[k] = np.ascontiguousarray(inputs[k], np.float32)
        if with_bias:
            for k in _BIAS_KEYS:
                m[k] = np.ascontiguousarray(np.asarray(inputs[k], np.float32).reshape(1, -1))
        per_core.append(m)
    return per_core


def _needs_bias(inputs):
    return any(np.abs(np.asarray(inputs[k])).max() > 0 for k in _BIAS_KEYS)


def kernel(**inputs):
    from concourse.bass_utils import run_bass_kernel_spmd

    with_bias = _needs_bias(inputs)
    nc = build(NG, with_bias)
    in_maps = _shard_inputs(inputs, with_bias, NCORE, NG)
    res = run_bass_kernel_spmd(nc, in_maps, core_ids=list(range(NCORE)))
    out = np.concatenate([r["out"] for r in res.results], axis=0)
    return np.ascontiguousarray(out, np.float32)
